# revision 2
# baseline (speedup 1.0000x reference)
"""Trainium2 Bass kernel for hierarchical softmax tree posterior (HNet.predict).

v2: fp16 row-pair packed tree multiply.

Math identities (same as baseline):
  softmax([l0,l1])[0] = sigmoid(l0-l1) => one [B,65] @ [65,4095] matmul
  (bias folded as 65th row), sigmoid on ScalarE, multiply-down-tree on DVE:
  child0 = p * s, child1 = p - child0.

Key perf change vs baseline: the tree multiply is DVE-bound (fp32
tensor_tensor = 1 elem/lane/cyc).  DVE gets 2x throughput for 16-bit
dtypes when every operand's innermost AP dim is a contiguous pair
(step +-1, count >=2, 4B aligned).  We therefore pack TWO batch rows
per fp16 pair: every post-PSUM tensor is laid out [128 part, ..., node,
2(row)] so ALL tree ops (including the stride-2 interleaved child
writes, which become outer dims) run in 2x_1p mode.  fp16 keeps
accumulated rounding ~1e-3 << 2e-2 budget (bf16 would be ~1e-2, too
close).  Output is stored/DMAd as fp16 (halves the 16MB/core output
traffic) in (pair, part, leaf, row) order; host deinterleaves+upcasts.

Sharding: batch B=8192 split across 8 cores (1024 rows each); tree
params replicated.
"""

import contextlib

import numpy as np

import concourse.bacc as bacc
import concourse.mybir as mybir
import concourse.tile as tile
from concourse.bass_utils import run_bass_kernel_spmd

B, D = 8192, 64
NODES = 4095          # internal nodes, level-order
LEAVES = 4096
NCORES = 8
BLOC = B // NCORES    # 1024 rows per core
KA = D + 1            # contraction dim incl. bias row
NBT = BLOC // 128     # 8 batch tiles of 128 rows
NBP = NBT // 2        # 4 row-pair tiles of 256 rows

F32 = mybir.dt.float32
F16 = mybir.dt.float16
MM_DT = mybir.dt.float32r


def _build(reps=1, do_compile=True):
    nc = bacc.Bacc("TRN2", target_bir_lowering=False, debug=False, num_devices=NCORES)
    wdt = nc.dram_tensor("wdt", [KA, LEAVES], MM_DT, kind="ExternalInput")
    xt = nc.dram_tensor("xt", [KA, BLOC], MM_DT, kind="ExternalInput")
    # out[bp*128+p, (leaf, e)]: batch row bp*256 + e*128 + p, fp16
    out = nc.dram_tensor("out", [NBP * 128, LEAVES * 2], F16, kind="ExternalOutput")

    SIG = mybir.ActivationFunctionType.Sigmoid
    IDN = mybir.ActivationFunctionType.Identity

    with tile.TileContext(nc) as tc:
        with (
            tc.tile_pool(name="const", bufs=1) as const,
            tc.tile_pool(name="pa", bufs=1) as pa,
            tc.tile_pool(name="pb", bufs=2) as pb,
            tc.tile_pool(name="ps", bufs=2, space="PSUM") as psp,
        ):
            wdt_r = const.tile([KA, LEAVES], MM_DT)
            xt_r = const.tile([KA, BLOC], MM_DT)
            nc.sync.dma_start(out=wdt_r[:], in_=wdt[:])
            nc.sync.dma_start(out=xt_r[:], in_=xt[:])

            loop = tc.For_i(0, reps, 1) if reps > 1 else contextlib.nullcontext()
            with loop:
                _emit_body(nc, tc, pa, pb, psp, wdt_r, xt_r, out, SIG, IDN)

    if do_compile:
        nc.compile()
    return nc


def _emit_body(nc, tc, pa, pb, psp, wdt_r, xt_r, out, SIG, IDN):
    # ---- phase A: nodes 0..1022 (levels 0..9), all 4 row-pairs fused.
    # s_small[p, bp, node, e] fp16; col 1023 is junk (node 1023 = level 10).
    s_small = pa.tile([128, NBP, 1024, 2], F16, tag="s_small")
    for bp in range(NBP):
        for e in range(2):
            bt = bp * 2 + e
            ps = psp.tile([128, 1024], F32, tag="ps")
            for c in range(2):
                nc.tensor.matmul(
                    ps[:, c * 512:(c + 1) * 512],
                    xt_r[:, bt * 128:(bt + 1) * 128],
                    wdt_r[:, c * 512:(c + 1) * 512],
                    start=True, stop=True,
                )
            nc.scalar.activation(out=s_small[:, bp, :, e], in_=ps[:], func=SIG)

    # Tree levels 0..9 on DVE (fp16 2x: innermost e-pair contiguous).
    pA = pa.tile([128, NBP, 512, 2], F16, tag="pA")
    pB = pa.tile([128, NBP, 512, 2], F16, tag="pB")
    p10 = pa.tile([128, NBP, 1024, 2], F16, tag="p10")
    # level 0: p1 = [s0, 1-s0] -> pA[:, :, 0:2, :]
    nc.vector.tensor_copy(pA[:, :, 0:1, :], s_small[:, :, 0:1, :])
    nc.scalar.activation(out=pA[:, :, 1:2, :], in_=s_small[:, :, 0:1, :],
                         func=IDN, bias=1.0, scale=-1.0)
    cur, other = pA, pB
    for lvl in range(1, 10):
        n = 1 << lvl
        off = n - 1
        nxt = p10 if lvl == 9 else other
        nxt4 = nxt[:, :, 0:2 * n, :].rearrange(
            "p g (n two) e -> p g n two e", two=2)
        nc.vector.tensor_mul(nxt4[:, :, :, 0, :], cur[:, :, 0:n, :],
                             s_small[:, :, off:off + n, :])
        nc.vector.tensor_sub(nxt4[:, :, :, 1, :], cur[:, :, 0:n, :],
                             nxt4[:, :, :, 0, :])
        other, cur = cur, nxt

    # ---- phase B: levels 10..11, per row-pair
    for bp in range(NBP):
        s10 = pb.tile([128, 1024, 2], F16, tag="s10")
        s11 = pb.tile([128, 2048, 2], F16, tag="s11")
        for e in range(2):
            bt = bp * 2 + e
            ps1 = psp.tile([128, 1024], F32, tag="ps")   # lvl-10 nodes 1023..2046
            for c in range(2):
                nc.tensor.matmul(
                    ps1[:, c * 512:(c + 1) * 512],
                    xt_r[:, bt * 128:(bt + 1) * 128],
                    wdt_r[:, 1023 + c * 512:1023 + (c + 1) * 512],
                    start=True, stop=True,
                )
            ps2 = psp.tile([128, 2048], F32, tag="ps")   # lvl-11 nodes 2047..4094
            for c in range(4):
                nc.tensor.matmul(
                    ps2[:, c * 512:(c + 1) * 512],
                    xt_r[:, bt * 128:(bt + 1) * 128],
                    wdt_r[:, 2047 + c * 512:2047 + (c + 1) * 512],
                    start=True, stop=True,
                )
            nc.scalar.activation(out=s10[:, :, e], in_=ps1[:], func=SIG)
            nc.scalar.activation(out=s11[:, :, e], in_=ps2[:], func=SIG)

        # level 10: p10[bp] [*,1024,2] -> p11 [*,2048,2]
        p11 = pb.tile([128, 2048, 2], F16, tag="p11")
        p11v = p11.rearrange("p (n two) e -> p n two e", two=2)
        nc.vector.tensor_mul(p11v[:, :, 0, :], p10[:, bp], s10[:])
        nc.vector.tensor_sub(p11v[:, :, 1, :], p10[:, bp], p11v[:, :, 0, :])

        # level 11: p11 [*,2048,2] -> ot [*, (2048 leafpair, 2 leaf), 2 row]
        ot = pb.tile([128, 4096, 2], F16, tag="out")
        otv = ot.rearrange("p (n two) e -> p n two e", two=2)
        nc.vector.tensor_mul(otv[:, :, 0, :], p11[:], s11[:])
        nc.vector.tensor_sub(otv[:, :, 1, :], p11[:], otv[:, :, 0, :])

        nc.sync.dma_start(
            out=out[bp * 128:(bp + 1) * 128, :],
            in_=ot.rearrange("p n e -> p (n e)"),
        )


_NC_CACHE = {}


def _get_nc(reps=1):
    if reps not in _NC_CACHE:
        _NC_CACHE[reps] = _build(reps)
    return _NC_CACHE[reps]


def _prep_inputs(x, W, b):
    x = np.asarray(x, dtype=np.float32)
    W = np.asarray(W, dtype=np.float32)
    b = np.asarray(b, dtype=np.float32)
    Wd = W[:, 0, :] - W[:, 1, :]          # [4095, 64]
    bd = b[:, 0] - b[:, 1]                # [4095]
    wdt = np.zeros((KA, LEAVES), dtype=np.float32)
    wdt[:D, :NODES] = Wd.T
    wdt[D, :NODES] = bd
    xt = np.empty((KA, B), dtype=np.float32)
    xt[:D] = x.T
    xt[D] = 1.0
    in_maps = [
        {"wdt": wdt, "xt": np.ascontiguousarray(xt[:, c * BLOC:(c + 1) * BLOC])}
        for c in range(NCORES)
    ]
    return in_maps


def _unpack_out(res):
    """[NBP*128, LEAVES*2] fp16 per core -> [B, LEAVES] f32."""
    parts = []
    for c in range(NCORES):
        a = res.results[c]["out"]                     # [512, 8192] fp16
        a = a.reshape(NBP, 128, LEAVES, 2)            # [bp, p, leaf, e]
        a = a.transpose(0, 3, 1, 2).reshape(BLOC, LEAVES)  # row bp*256+e*128+p
        parts.append(a)
    return np.concatenate(parts, axis=0).astype(np.float32)


def kernel(x, W, b):
    in_maps = _prep_inputs(x, W, b)
    nc = _get_nc()
    res = run_bass_kernel_spmd(nc, in_maps, core_ids=list(range(NCORES)))
    return _unpack_out(res)


if __name__ == "__main__":
    rng = np.random.default_rng(0)
    x = rng.standard_normal((B, D)).astype(np.float32)
    W = (rng.standard_normal((NODES, 2, D)) * 0.1).astype(np.float32)
    b = (rng.standard_normal((NODES, 2)) * 0.1).astype(np.float32)
    p = kernel(x, W, b)
    print("out", p.shape, p.dtype, "rowsum", p.sum(axis=1)[:4])


# revision 3
# speedup vs baseline: 5.5333x; 5.5333x over previous
"""Trainium2 Bass kernel for hierarchical softmax tree posterior (HNet.predict).

Math: per internal node i (level-order, children 2i+1/2i+2), softmax over 2
children of Linear(x). Path probabilities multiply down a depth-12 complete
binary tree; output p [B, 4096] leaf posteriors.

Key identities used:
  softmax([l0, l1])[0] = sigmoid(l0 - l1), [1] = 1 - sigmoid(l0 - l1)
  => only the logit DIFFERENCE matters: d_j = x . (W_j0 - W_j1) + (b_j0 - b_j1)
  => one [B,64] @ [64,4095] matmul (bias folded in as a 65th contraction row),
     sigmoid on ScalarE, then multiply-down-the-tree on VectorE:
     child0 = p * s, child1 = p - child0.
     (GPSIMD offload of subtractions was measured NET-NEGATIVE: it shares an
     SBUF port with VectorE and serializes; TensorTensor cannot run on
     ScalarE on TRN2 — so the whole tree stays on the DVE.)

Sharding: batch B=8192 split across 8 cores (1024 rows each); tree params
replicated. Output [B, 4096] f32 = 128MB dominates traffic (memory-bound).
"""

import contextlib

import numpy as np

import concourse.bacc as bacc
import concourse.mybir as mybir
import concourse.tile as tile
from concourse.bass_utils import run_bass_kernel_spmd

B, D = 8192, 64
NODES = 4095          # internal nodes, level-order
LEAVES = 4096
NCORES = 8
BLOC = B // NCORES    # 1024 rows per core
KA = D + 1            # contraction dim incl. bias row
NBT = BLOC // 128     # 8 batch tiles of 128 rows

F32 = mybir.dt.float32
# float32r runs the PE at 1 cyc/row (vs 4 for exact fp32); measured end-to-end
# output error 2.4e-4 rel-to-scale. DRAM inputs are declared float32r directly
# (same bytes as f32) so no on-device cast is needed.
MM_DT = mybir.dt.float32r

# Pair-columns of the level-10/11 odd-child subtractions on GPSIMD instead of
# VectorE. Measured on HW: any GPSIMD share is slower (shared SBUF port with
# DVE serializes the engines), so these stay 0.
GP_SUB10 = 0      # of 1024
GP_SUB11 = 0      # of 2048


def _build(reps=1):
    nc = bacc.Bacc("TRN2", target_bir_lowering=False, debug=False, num_devices=NCORES)
    wdt = nc.dram_tensor("wdt", [KA, LEAVES], MM_DT, kind="ExternalInput")
    xt = nc.dram_tensor("xt", [KA, BLOC], MM_DT, kind="ExternalInput")
    out = nc.dram_tensor("out", [BLOC, LEAVES], F32, kind="ExternalOutput")

    SIG = mybir.ActivationFunctionType.Sigmoid
    IDN = mybir.ActivationFunctionType.Identity

    with tile.TileContext(nc) as tc:
        with (
            tc.tile_pool(name="const", bufs=1) as const,
            tc.tile_pool(name="pa", bufs=1) as pa,
            tc.tile_pool(name="pb", bufs=2) as pb,
            tc.tile_pool(name="ps", bufs=2, space="PSUM") as psp,
        ):
            wdt_r = const.tile([KA, LEAVES], MM_DT)
            xt_r = const.tile([KA, BLOC], MM_DT)
            nc.sync.dma_start(out=wdt_r[:], in_=wdt[:])
            nc.sync.dma_start(out=xt_r[:], in_=xt[:])

            loop = tc.For_i(0, reps, 1) if reps > 1 else contextlib.nullcontext()
            with loop:
                _emit_body(nc, tc, pa, pb, psp, wdt_r, xt_r, out, SIG, IDN)

    nc.compile()
    return nc


def _emit_body(nc, tc, pa, pb, psp, wdt_r, xt_r, out, SIG, IDN):
    # ---- phase A: nodes 0..1022 (levels 0..9) fused across all 8 batch tiles
    s_small = pa.tile([128, NBT, 1024], F32, tag="s_small")
    for bt in range(NBT):
        ps = psp.tile([128, 1024], F32, tag="ps")
        for c in range(2):
            nc.tensor.matmul(
                ps[:, c * 512:(c + 1) * 512],
                xt_r[:, bt * 128:(bt + 1) * 128],
                wdt_r[:, c * 512:(c + 1) * 512],
                start=True, stop=True,
            )
        nc.scalar.activation(out=s_small[:, bt, :], in_=ps[:], func=SIG)

    pA = pa.tile([128, NBT, 512], F32, tag="pA")
    pB = pa.tile([128, NBT, 512], F32, tag="pB")
    p10 = pa.tile([128, NBT, 1024], F32, tag="p10")
    # level 0: p1 = [s0, 1-s0]
    nc.vector.tensor_copy(pA[:, :, 0:1], s_small[:, :, 0:1])
    nc.scalar.activation(out=pA[:, :, 1:2], in_=s_small[:, :, 0:1],
                         func=IDN, bias=1.0, scale=-1.0)
    cur, other = pA, pB
    for lvl in range(1, 10):
        n = 1 << lvl
        off = n - 1
        nxt = p10 if lvl == 9 else other
        nxt4 = nxt[:, :, 0:2 * n].rearrange("p g (n two) -> p g n two", two=2)
        nc.vector.tensor_mul(nxt4[:, :, :, 0], cur[:, :, 0:n],
                             s_small[:, :, off:off + n])
        nc.vector.tensor_sub(nxt4[:, :, :, 1], cur[:, :, 0:n],
                             nxt4[:, :, :, 0])
        other, cur = cur, nxt

    # ---- phase B: nodes 1023..4094 (levels 10..11), per batch tile
    for bt in range(NBT):
        ps1 = psp.tile([128, 2048], F32, tag="ps")   # nodes 1023..3070
        for c in range(4):
            nc.tensor.matmul(
                ps1[:, c * 512:(c + 1) * 512],
                xt_r[:, bt * 128:(bt + 1) * 128],
                wdt_r[:, 1023 + c * 512:1023 + (c + 1) * 512],
                start=True, stop=True,
            )
        ps2 = psp.tile([128, 1024], F32, tag="ps")   # nodes 3071..4094
        for c in range(2):
            nc.tensor.matmul(
                ps2[:, c * 512:(c + 1) * 512],
                xt_r[:, bt * 128:(bt + 1) * 128],
                wdt_r[:, 3071 + c * 512:3071 + (c + 1) * 512],
                start=True, stop=True,
            )
        sb = pb.tile([128, 1024], F32, tag="sbig")
        nc.scalar.activation(out=sb[:], in_=ps1[:, 0:1024], func=SIG)
        # level-11 sigmoids written interleaved (sigma(+d), sigma(-d)) so the
        # last level needs only ONE DVE multiply with a step-0 broadcast of
        # p11 and a unit-stride output (replaces strided mul+sub pair).
        s11 = pb.tile([128, 4096], F32, tag="s11")
        s11v = s11.rearrange("p (n two) -> p n two", two=2)
        nc.scalar.activation(out=s11v[:, 0:1024, 0], in_=ps1[:, 1024:2048], func=SIG)
        nc.scalar.activation(out=s11v[:, 0:1024, 1], in_=ps1[:, 1024:2048], func=SIG, scale=-1.0)
        nc.scalar.activation(out=s11v[:, 1024:2048, 0], in_=ps2[:], func=SIG)
        nc.scalar.activation(out=s11v[:, 1024:2048, 1], in_=ps2[:], func=SIG, scale=-1.0)

        # level 10: p10 [*,1024] -> p11 [*,2048]; s nodes 1023..2046
        p11 = pb.tile([128, 2048], F32, tag="p11")
        p11v = p11.rearrange("p (n two) -> p n two", two=2)
        nc.vector.tensor_mul(p11v[:, :, 0], p10[:, bt, :], sb[:])
        nc.vector.tensor_sub(p11v[:, :, 1], p10[:, bt, :], p11v[:, :, 0])

        # level 11: one broadcast multiply into the output tile
        ot = pb.tile([128, 4096], F32, tag="out")
        otv = ot.rearrange("p (n two) -> p n two", two=2)
        nc.vector.tensor_mul(otv[:], p11[:].broadcast_to([128, 2048, 2]), s11v[:])

        nc.sync.dma_start(out=out[bt * 128:(bt + 1) * 128, :], in_=ot[:])


_NC_CACHE = {}


def _get_nc(reps=1):
    if reps not in _NC_CACHE:
        _NC_CACHE[reps] = _build(reps)
    return _NC_CACHE[reps]


def _prep_inputs(x, W, b):
    x = np.asarray(x, dtype=np.float32)
    W = np.asarray(W, dtype=np.float32)
    b = np.asarray(b, dtype=np.float32)
    Wd = W[:, 0, :] - W[:, 1, :]          # [4095, 64]
    bd = b[:, 0] - b[:, 1]                # [4095]
    wdt = np.zeros((KA, LEAVES), dtype=np.float32)
    wdt[:D, :NODES] = Wd.T
    wdt[D, :NODES] = bd
    xt = np.empty((KA, B), dtype=np.float32)
    xt[:D] = x.T
    xt[D] = 1.0
    in_maps = [
        {"wdt": wdt, "xt": np.ascontiguousarray(xt[:, c * BLOC:(c + 1) * BLOC])}
        for c in range(NCORES)
    ]
    return in_maps


def kernel(x, W, b):
    in_maps = _prep_inputs(x, W, b)
    nc = _get_nc()
    res = run_bass_kernel_spmd(nc, in_maps, core_ids=list(range(NCORES)))
    return np.concatenate([res.results[c]["out"] for c in range(NCORES)], axis=0)


if __name__ == "__main__":
    rng = np.random.default_rng(0)
    x = rng.standard_normal((B, D)).astype(np.float32)
    W = (rng.standard_normal((NODES, 2, D)) * 0.1).astype(np.float32)
    b = (rng.standard_normal((NODES, 2)) * 0.1).astype(np.float32)
    p = kernel(x, W, b)
    print("out", p.shape, p.dtype, "rowsum", p.sum(axis=1)[:4])

